# revision 17
# baseline (speedup 1.0000x reference)
"""Trainium2 Bass kernel: gate-merged tensor-train (TT) MoE layer.

Reference math (per batch element b):
    merge each TT core over experts with gates[b]  ->  C_i  (tiny, <=512 floats)
    then an 8-step TT contraction maps X[b] : [512, 4096] -> Y[b] : [512, 4096].

Because every boundary rank is 8, the whole chain collapses exactly to a
rank-8 factorization:
    Y_b = X_b @ W4_b @ E4_b
with W4_b = C0*C1*C2*C3 reshaped to [4096, 8] and E4_b = C4*C5*C6*C7
reshaped to [8, 4096].  The factor merge is ~0.5 MFLOP of 8x8-sized einsums
(done on host at float64); all heavy lifting (16 MiB of streaming, the
4096-wide contractions) runs on the NeuronCores.

Sharding: data-parallel over batch B=8 across the 8 cores (one batch
element per core); the merged factors are per-core constants.

On-device dataflow per core:
    X  --DMA-->  SBUF (4 s-chunks of [128, 4096])
    PE transpose (fp32, identity matmul) -> PSUM -> ACT copy -> XT (feat-major)
    mm1 (f32r):  t4[8, s] += W4_chunk^T @ XT_chunk  (32 K=128 chunks, PSUM acc)
    mm2 (f32r):  Y[s-chunk, n-chunk] = t4_chunk^T @ E4  -> PSUM -> DVE -> SBUF
    DMA out, row-contiguous.
"""

import numpy as np

B = 8
S = 512
F = 4096
R = 8
N_CORES = 8

_PROGRAM = None
LAST_RESULTS = None  # BassKernelResults of the most recent run (for profiling)
TRACE = False        # set True (e.g. from test.py) to capture an NTFF profile


def _merge_factors(gates, core_first, cores_mid, core_last):
    """Host-side merge of the tiny TT cores -> per-batch rank-8 factors.

    Returns (W4, E4): [B, 4096, 8] and [B, 8, 4096] float32.
    """
    g = np.asarray(gates, np.float64)
    cf = np.asarray(core_first, np.float64)   # [E, 1, 8, 8]
    cm = np.asarray(cores_mid, np.float64)    # [6, E, 8, 8, 8]
    cl = np.asarray(core_last, np.float64)    # [E, 8, 8, 1]

    W4 = np.empty((B, F, R), np.float32)
    E4 = np.empty((B, R, F), np.float32)
    for b in range(B):
        C = [np.einsum('e,ermp->rmp', g[b], cf)] + \
            [np.einsum('e,ermp->rmp', g[b], cm[i]) for i in range(6)] + \
            [np.einsum('e,ermp->rmp', g[b], cl)]
        C0 = C[0][0]                                  # [m0, p1]
        W = np.einsum('ab,bic->iac', C0, C[1])        # [m1, m0, p2]
        W = np.einsum('iac,cjd->jiad', W, C[2])       # [m2, m1, m0, p3]
        W = np.einsum('jiad,dke->kjiae', W, C[3])     # [m3, m2, m1, m0, p4]
        W4[b] = W.reshape(F, R)
        E = np.einsum('anb,bmc->anmc', C[4], C[5])    # [p4, n0, n1, p6]
        E = np.einsum('anmc,ckd->anmkd', E, C[6])     # [p4, n0, n1, n2, p7]
        E = np.einsum('anmkd,dl->anmkl', E, C[7][:, :, 0])
        E4[b] = E.reshape(R, F)
    return W4, E4


def _build_program():
    """Build + compile the per-core Bass/Tile program (identical on all cores).

    Pipeline (per s-quarter c of 128 seq positions):
      x[c] --HWDGE DMA (fp32)--> SBUF --GpSimd copy--> fp16
      PE: 32 transpose-matmuls (xc_slice^T @ I, exact) -> PSUM
      ACT/DVE (alternating): drain PSUM -> xt_q[c]  (feat-major, fp16)
      PE: mm1 = 32 accumulating matmuls -> t4 [8, 128]; ACT -> fp16
      PE: mm2 = 8 matmuls t4^T @ E4 -> PSUM; ACT/DVE drain -> orow halves
      HWDGE DMA orow halves -> y[c]
    All matmuls fp16 (1 cycle/row, FWL weight loads, fp32 PSUM accumulate);
    the host splits a power-of-2 scale between W4 and E4 so every fp16
    operand stays well inside range.
    """
    from contextlib import ExitStack
    import concourse.bass as bass
    import concourse.tile as tile
    from concourse import bacc, mybir

    f32 = mybir.dt.float32
    f16 = mybir.dt.float16

    nc = bacc.Bacc("TRN2", target_bir_lowering=False, debug=False)
    x_d = nc.dram_tensor("x", [S, F], f32, kind="ExternalInput").ap()
    # w4 comes pre-swizzled from host: w4[p, 8*f + m] = SCALE*W4[128*f + p, m]
    w4_d = nc.dram_tensor("w4", [128, 256], f16, kind="ExternalInput").ap()
    e4_d = nc.dram_tensor("e4", [R, F], f16, kind="ExternalInput").ap()
    id_d = nc.dram_tensor("ident", [128, 128], f16, kind="ExternalInput").ap()
    y_d = nc.dram_tensor("y", [S, F], f32, kind="ExternalOutput").ap()

    with tile.TileContext(nc) as tc, ExitStack() as ctx:
        const = ctx.enter_context(tc.tile_pool(name="const", bufs=1))
        x16p = ctx.enter_context(tc.tile_pool(name="x16", bufs=1))
        xtp = ctx.enter_context(tc.tile_pool(name="xt", bufs=1))
        t4p = ctx.enter_context(tc.tile_pool(name="t4", bufs=1))
        outp = ctx.enter_context(tc.tile_pool(name="out", bufs=2))
        ps_tr = ctx.enter_context(
            tc.tile_pool(name="ps_tr", bufs=3, space=bass.MemorySpace.PSUM))
        ps_t4 = ctx.enter_context(
            tc.tile_pool(name="ps_t4", bufs=1, space=bass.MemorySpace.PSUM))
        ps_out = ctx.enter_context(
            tc.tile_pool(name="ps_out", bufs=4, space=bass.MemorySpace.PSUM))

        # All X loads issue up front on the SWDGE (gpsimd) ring in consumption
        # order, casting fp32 -> fp16 inline in the DMA datapath.  The HWDGE
        # (sync) ring carries consts + output stores so neither blocks the
        # other.
        xc = {}
        for c in range(4):
            for a in range(2):
                t = x16p.tile([128, F // 2], f16, tag=f"xc{c}{a}",
                              name=f"xc{c}{a}")
                nc.gpsimd.dma_start(
                    t[:], x_d[c * 128:(c + 1) * 128,
                              a * (F // 2):(a + 1) * (F // 2)])
                xc[c, a] = t

        w4 = const.tile([128, 256], f16, tag="w4")
        # e4 replicated into four partition bands (32i..32i+8) so mm2 can
        # run 4 row-group-packed matmuls concurrently on the PE.
        e4 = const.tile([128, F], f16, tag="e4")
        ident = const.tile([128, 128], f16, tag="ident")
        nc.sync.dma_start(w4[:], w4_d)
        for i in range(4):
            nc.sync.dma_start(e4[32 * i:32 * i + R, :], e4_d)
        nc.sync.dma_start(ident[:], id_d)

        # Per-quarter transposed X: xt_q[c][p, 128*f + s] = X[128*c + s, 128*f + p]
        xt_q = [xtp.tile([128, 32 * 128], f16, tag=f"xtq{c}", name=f"xtq{c}")
                for c in range(4)]
        t4_q = [t4p.tile([128, 128], f16, tag=f"t4q{c}", name=f"t4q{c}")
                for c in range(4)]

        def transpose_quarter(c):
            # out = xc_slice^T @ I via regular fp16 matmuls: exact (x*1), and
            # unlike transpose-mode these warm the PE clock gate (HAM).
            for fg in range(8):          # 4 feat-chunks share a 1-bank tile
                pt = ps_tr.tile([128, 512], f32, tag="pt")
                for j in range(4):
                    f = fg * 4 + j
                    src = xc[c, f // 16]
                    fo = f % 16
                    nc.tensor.matmul(
                        pt[:, j * 128:(j + 1) * 128],
                        src[:, fo * 128:(fo + 1) * 128],
                        ident[:],
                        start=True, stop=True,
                    )
                dst = xt_q[c][:, fg * 512:(fg + 1) * 512]
                if fg % 2 == 0:
                    nc.scalar.copy(dst, pt[:])
                else:
                    nc.vector.tensor_copy(dst, pt[:])

        def mm1(c):
            acc = ps_t4.tile([R, 128], f32, tag="pacc")
            xt = xt_q[c]
            for f in range(32):
                nc.tensor.matmul(
                    acc[:],
                    w4[:, f * 8:(f + 1) * 8],
                    xt[:, f * 128:(f + 1) * 128],
                    start=(f == 0),
                    stop=(f == 31),
                )
            for i in range(4):
                dst = t4_q[c][32 * i:32 * i + R, :]
                if i % 2 == 0:
                    nc.scalar.copy(dst, acc[:])
                else:
                    nc.vector.tensor_copy(dst, acc[:])

        def mm2_store(c):
            # Output row in two half tiles so each 1 MiB store can start
            # after 4 drains; drains alternate DVE/ACT.
            for a in range(2):
                orow = outp.tile([128, F // 2], f32, tag=f"orow{a}",
                                 name=f"orow{a}")
                pos = []
                for k in range(4):
                    n = a * 4 + k
                    po = ps_out.tile([128, 512], f32, tag="po")
                    nc.tensor.matmul(
                        po[:],
                        t4_q[c][32 * k:32 * k + R, :],
                        e4[32 * k:32 * k + R, n * 512:(n + 1) * 512],
                        start=True,
                        stop=True,
                        tile_position=(32 * k, 0),
                    )
                    pos.append(po)
                for k in range(4):
                    n = a * 4 + k
                    dst = orow[:, k * 512:(k + 1) * 512]
                    if n % 2 == 0:
                        nc.vector.tensor_copy(dst, pos[k][:])
                    else:
                        nc.scalar.copy(dst, pos[k][:])
                nc.sync.dma_start(
                    y_d[c * 128:(c + 1) * 128,
                        a * (F // 2):(a + 1) * (F // 2)], orow[:])

        for c in range(4):
            transpose_quarter(c)
            mm1(c)
            mm2_store(c)

    nc.compile()
    return nc


def _get_program():
    global _PROGRAM
    if _PROGRAM is None:
        _PROGRAM = _build_program()
    return _PROGRAM


SCALE = 0.125  # power-of-2 split keeps every fp16 operand >=5x under 65504


def _make_in_maps(X, gates, core_first, cores_mid, core_last):
    W4, E4 = _merge_factors(gates, core_first, cores_mid, core_last)
    ident = np.eye(128, dtype=np.float16)
    X = np.ascontiguousarray(np.asarray(X, np.float32))
    in_maps = []
    for b in range(B):
        w4sb = np.ascontiguousarray(
            (W4[b] * SCALE).reshape(32, 128, R).transpose(1, 0, 2)
            .reshape(128, 256).astype(np.float16))
        in_maps.append({
            "x": X[b],
            "w4": w4sb,
            "e4": np.ascontiguousarray((E4[b] / SCALE).astype(np.float16)),
            "ident": ident,
        })
    return in_maps


def kernel(X, gates, core_first, cores_mid, core_last):
    global LAST_RESULTS
    from concourse.bass_utils import run_bass_kernel_spmd

    nc = _get_program()
    in_maps = _make_in_maps(X, gates, core_first, cores_mid, core_last)
    res = run_bass_kernel_spmd(nc, in_maps, list(range(N_CORES)), trace=TRACE)
    LAST_RESULTS = res
    Y = np.stack([res.results[b]["y"] for b in range(B)], axis=0)
    return Y.astype(np.float32, copy=False)


# revision 18
# speedup vs baseline: 1.1671x; 1.1671x over previous
"""Trainium2 Bass kernel: gate-merged tensor-train (TT) MoE layer.

Reference math (per batch element b):
    merge each TT core over experts with gates[b]  ->  C_i  (tiny, <=512 floats)
    then an 8-step TT contraction maps X[b] : [512, 4096] -> Y[b] : [512, 4096].

Because every boundary rank is 8, the whole chain collapses exactly to a
rank-8 factorization:
    Y_b = X_b @ W4_b @ E4_b
with W4_b = C0*C1*C2*C3 reshaped to [4096, 8] and E4_b = C4*C5*C6*C7
reshaped to [8, 4096].  The factor merge is ~0.5 MFLOP of 8x8-sized einsums
(done on host at float64); all heavy lifting (16 MiB of streaming, the
4096-wide contractions) runs on the NeuronCores.

Sharding: data-parallel over batch B=8 across the 8 cores (one batch
element per core); the merged factors are per-core constants.

On-device dataflow per core:
    X  --DMA-->  SBUF (4 s-chunks of [128, 4096])
    PE transpose (fp32, identity matmul) -> PSUM -> ACT copy -> XT (feat-major)
    mm1 (f32r):  t4[8, s] += W4_chunk^T @ XT_chunk  (32 K=128 chunks, PSUM acc)
    mm2 (f32r):  Y[s-chunk, n-chunk] = t4_chunk^T @ E4  -> PSUM -> DVE -> SBUF
    DMA out, row-contiguous.
"""

import numpy as np

B = 8
S = 512
F = 4096
R = 8
N_CORES = 8

_PROGRAM = None
LAST_RESULTS = None  # BassKernelResults of the most recent run (for profiling)
TRACE = False        # set True (e.g. from test.py) to capture an NTFF profile


def _merge_factors(gates, core_first, cores_mid, core_last):
    """Host-side merge of the tiny TT cores -> per-batch rank-8 factors.

    Returns (W4, E4): [B, 4096, 8] and [B, 8, 4096] float32.
    """
    g = np.asarray(gates, np.float64)
    cf = np.asarray(core_first, np.float64)   # [E, 1, 8, 8]
    cm = np.asarray(cores_mid, np.float64)    # [6, E, 8, 8, 8]
    cl = np.asarray(core_last, np.float64)    # [E, 8, 8, 1]

    W4 = np.empty((B, F, R), np.float32)
    E4 = np.empty((B, R, F), np.float32)
    for b in range(B):
        C = [np.einsum('e,ermp->rmp', g[b], cf)] + \
            [np.einsum('e,ermp->rmp', g[b], cm[i]) for i in range(6)] + \
            [np.einsum('e,ermp->rmp', g[b], cl)]
        C0 = C[0][0]                                  # [m0, p1]
        W = np.einsum('ab,bic->iac', C0, C[1])        # [m1, m0, p2]
        W = np.einsum('iac,cjd->jiad', W, C[2])       # [m2, m1, m0, p3]
        W = np.einsum('jiad,dke->kjiae', W, C[3])     # [m3, m2, m1, m0, p4]
        W4[b] = W.reshape(F, R)
        E = np.einsum('anb,bmc->anmc', C[4], C[5])    # [p4, n0, n1, p6]
        E = np.einsum('anmc,ckd->anmkd', E, C[6])     # [p4, n0, n1, n2, p7]
        E = np.einsum('anmkd,dl->anmkl', E, C[7][:, :, 0])
        E4[b] = E.reshape(R, F)
    return W4, E4


def _build_program():
    """Build + compile the per-core Bass/Tile program (identical on all cores).

    Pipeline (per s-quarter c of 128 seq positions):
      x[c] --HWDGE DMA (fp32)--> SBUF --GpSimd copy--> fp16
      PE: 32 transpose-matmuls (xc_slice^T @ I, exact) -> PSUM
      ACT/DVE (alternating): drain PSUM -> xt_q[c]  (feat-major, fp16)
      PE: mm1 = 32 accumulating matmuls -> t4 [8, 128]; ACT -> fp16
      PE: mm2 = 8 matmuls t4^T @ E4 -> PSUM; ACT/DVE drain -> orow halves
      HWDGE DMA orow halves -> y[c]
    All matmuls fp16 (1 cycle/row, FWL weight loads, fp32 PSUM accumulate);
    the host splits a power-of-2 scale between W4 and E4 so every fp16
    operand stays well inside range.
    """
    from contextlib import ExitStack
    import concourse.bass as bass
    import concourse.tile as tile
    from concourse import bacc, mybir

    f32 = mybir.dt.float32
    f16 = mybir.dt.float16

    nc = bacc.Bacc("TRN2", target_bir_lowering=False, debug=False)
    x_d = nc.dram_tensor("x", [S, F], f32, kind="ExternalInput").ap()
    # w4 comes pre-swizzled from host: w4[p, 8*f + m] = SCALE*W4[128*f + p, m]
    w4_d = nc.dram_tensor("w4", [128, 256], f16, kind="ExternalInput").ap()
    e4_d = nc.dram_tensor("e4", [R, F], f16, kind="ExternalInput").ap()
    id_d = nc.dram_tensor("ident", [128, 128], f16, kind="ExternalInput").ap()
    y_d = nc.dram_tensor("y", [S, F], f32, kind="ExternalOutput").ap()

    with tile.TileContext(nc) as tc, ExitStack() as ctx:
        const = ctx.enter_context(tc.tile_pool(name="const", bufs=1))
        x16p = ctx.enter_context(tc.tile_pool(name="x16", bufs=1))
        xtp = ctx.enter_context(tc.tile_pool(name="xt", bufs=1))
        t4p = ctx.enter_context(tc.tile_pool(name="t4", bufs=1))
        outp = ctx.enter_context(tc.tile_pool(name="out", bufs=2))
        ps_tr = ctx.enter_context(
            tc.tile_pool(name="ps_tr", bufs=3, space=bass.MemorySpace.PSUM))
        ps_t4 = ctx.enter_context(
            tc.tile_pool(name="ps_t4", bufs=2, space=bass.MemorySpace.PSUM))
        ps_out = ctx.enter_context(
            tc.tile_pool(name="ps_out", bufs=3, space=bass.MemorySpace.PSUM))

        # All X loads issue up front on the SWDGE (gpsimd) ring in consumption
        # order, casting fp32 -> fp16 inline in the DMA datapath.  The HWDGE
        # (sync) ring carries consts + output stores so neither blocks the
        # other.
        xc = {}
        for c in range(4):
            for a in range(2):
                t = x16p.tile([128, F // 2], f16, tag=f"xc{c}{a}",
                              name=f"xc{c}{a}")
                nc.gpsimd.dma_start(
                    t[:], x_d[c * 128:(c + 1) * 128,
                              a * (F // 2):(a + 1) * (F // 2)])
                xc[c, a] = t

        w4 = const.tile([128, 256], f16, tag="w4")
        e4 = const.tile([R, F], f16, tag="e4")
        ident = const.tile([128, 128], f16, tag="ident")
        nc.sync.dma_start(ident[:], id_d)
        nc.sync.dma_start(w4[:], w4_d)
        nc.sync.dma_start(e4[:], e4_d)

        # Per-quarter transposed X: xt_q[c][p, 128*f + s] = X[128*c + s, 128*f + p]
        xt_q = [xtp.tile([128, 32 * 128], f16, tag=f"xtq{c}", name=f"xtq{c}")
                for c in range(4)]
        t4_q = [t4p.tile([R, 128], f16, tag=f"t4q{c}", name=f"t4q{c}")
                for c in range(4)]

        def transpose_quarter(c):
            # out = xc_slice^T @ I via regular fp16 matmuls: exact (x*1), and
            # unlike transpose-mode these warm the PE clock gate (HAM).
            for fg in range(8):          # 4 feat-chunks share a 1-bank tile
                pt = ps_tr.tile([128, 512], f32, tag="pt")
                for j in range(4):
                    f = fg * 4 + j
                    src = xc[c, f // 16]
                    fo = f % 16
                    nc.tensor.matmul(
                        pt[:, j * 128:(j + 1) * 128],
                        src[:, fo * 128:(fo + 1) * 128],
                        ident[:],
                        start=True, stop=True,
                    )
                dst = xt_q[c][:, fg * 512:(fg + 1) * 512]
                if fg % 2 == 0:
                    nc.scalar.copy(dst, pt[:])
                else:
                    nc.vector.tensor_copy(dst, pt[:])

        def mm1(c):
            acc = ps_t4.tile([R, 128], f32, tag="pacc")
            xt = xt_q[c]
            for f in range(32):
                nc.tensor.matmul(
                    acc[:],
                    w4[:, f * 8:(f + 1) * 8],
                    xt[:, f * 128:(f + 1) * 128],
                    start=(f == 0),
                    stop=(f == 31),
                )
            nc.scalar.copy(t4_q[c][:], acc[:])

        def mm2_store(c):
            # Output row in two half tiles so each 1 MiB store can start
            # after 4 drains; drains alternate DVE/ACT.
            for a in range(2):
                orow = outp.tile([128, F // 2], f32, tag=f"orow{a}",
                                 name=f"orow{a}")
                for k in range(4):
                    n = a * 4 + k
                    po = ps_out.tile([128, 512], f32, tag="po")
                    nc.tensor.matmul(
                        po[:],
                        t4_q[c][:],
                        e4[:, n * 512:(n + 1) * 512],
                        start=True,
                        stop=True,
                    )
                    dst = orow[:, k * 512:(k + 1) * 512]
                    if n % 2 == 0:
                        nc.vector.tensor_copy(dst, po[:])
                    else:
                        nc.scalar.copy(dst, po[:])
                nc.sync.dma_start(
                    y_d[c * 128:(c + 1) * 128,
                        a * (F // 2):(a + 1) * (F // 2)], orow[:])

        for c in range(4):
            transpose_quarter(c)
            mm1(c)
            mm2_store(c)

    nc.compile()
    return nc


def _get_program():
    global _PROGRAM
    if _PROGRAM is None:
        _PROGRAM = _build_program()
    return _PROGRAM


SCALE = 0.125  # power-of-2 split keeps every fp16 operand >=5x under 65504


def _make_in_maps(X, gates, core_first, cores_mid, core_last):
    W4, E4 = _merge_factors(gates, core_first, cores_mid, core_last)
    ident = np.eye(128, dtype=np.float16)
    X = np.ascontiguousarray(np.asarray(X, np.float32))
    in_maps = []
    for b in range(B):
        w4sb = np.ascontiguousarray(
            (W4[b] * SCALE).reshape(32, 128, R).transpose(1, 0, 2)
            .reshape(128, 256).astype(np.float16))
        in_maps.append({
            "x": X[b],
            "w4": w4sb,
            "e4": np.ascontiguousarray((E4[b] / SCALE).astype(np.float16)),
            "ident": ident,
        })
    return in_maps


def kernel(X, gates, core_first, cores_mid, core_last):
    global LAST_RESULTS
    from concourse.bass_utils import run_bass_kernel_spmd

    nc = _get_program()
    in_maps = _make_in_maps(X, gates, core_first, cores_mid, core_last)
    res = run_bass_kernel_spmd(nc, in_maps, list(range(N_CORES)), trace=TRACE)
    LAST_RESULTS = res
    Y = np.stack([res.results[b]["y"] for b in range(B)], axis=0)
    return Y.astype(np.float32, copy=False)
